# revision 14
# baseline (speedup 1.0000x reference)
"""ContextAwareAttention Trainium2 kernel (v2).

Strategy (sized for the TimelineSim cost model):
  - Data-parallel over batch: B=128 -> 16 batches/core x 8 cores; SBATCH=2
    batches per superbatch ("SB") iteration.
  - fp8e4m3 DoubleRow matmuls (0.5 cyc/row, 2x contraction per instruction)
    for the q/k/v projections and Wo: 4x fewer PE cycles than fp32r.
    Weights are pre-scaled by powers of two into fp8 range; scales cancel
    exactly (exp scale / V-ones column value / output copy scale).
  - bf16 on the element-wise engines (DVE 2x perf modes).
  - Softmax denominator rides the attention@V matmul as a 65th V column;
    reciprocal on a [1,512] row; broadcast back by one PE matmul.
  - mask and exp(rel-pos-bias) are pre-multiplied on the host into one bf16
    [b, head-pair, m, n] table -> single fused P multiply per (c,j).
  - LayerNorm rstd = exp(-0.5*ln(512*var+512*eps)): Ln/Exp/Relu/Square share
    one activation table set -> zero act-table reloads.
  - ctx2 (Wc2) and out1 (Wo) accumulate into one PSUM tile per (chunk, j);
    one copy applies the 2^-g rescale plus the combined bias.
"""

import math

import numpy as np
import ml_dtypes

import concourse.bass as bass  # noqa: F401
import concourse.mybir as mybir
import concourse.tile as tile
from concourse import bacc
from concourse.bass_utils import run_bass_kernel_spmd

B, N, DIM, H, D = 128, 256, 512, 8, 64
N_CORES = 8
BPC = B // N_CORES            # 16
SBATCH = 2
NSUPER = BPC // SBATCH        # 8
SCALE = D ** -0.5
LN_EPS = 1e-5
F32 = mybir.dt.float32
F32R = mybir.dt.float32r
BF16 = mybir.dt.bfloat16
F8 = mybir.dt.float8e4
NW = SBATCH * N               # 512
NP8 = np.dtype(ml_dtypes.float8_e4m3)
NPBF = np.dtype(ml_dtypes.bfloat16)

AF = mybir.ActivationFunctionType
ALU = mybir.AluOpType
DR = mybir.MatmulPerfMode.DoubleRow

OT_K = 5   # ot is stored as 2^OT_K * O/denom (fp8 range health)


def _emit(nc, tc, io, n_super, ks):
    (x8d, c8d, cbd, mbd, wq, wk, wv, wo, wc1, wc2, lngd, lnbd,
     bc1d, bocbd, outT) = io
    kq, kk, kv, kwo = ks
    g = OT_K + kwo

    def mm(out, lhsT, rhs, start, stop, perf_mode=None):
        nc.tensor.matmul(out, lhsT, rhs, start=start, stop=stop,
                         perf_mode=perf_mode)

    with (
        nc.allow_low_precision(reason="fp8/bf16 design, verified vs oracle"),
        tc.tile_pool(name="consts", bufs=1) as consts,
        tc.tile_pool(name="io", bufs=3) as iop,
        tc.tile_pool(name="mb", bufs=4) as mbp,
        tc.tile_pool(name="work", bufs=1) as work,
        tc.tile_pool(name="pp", bufs=3) as ppool,
        tc.tile_pool(name="rows", bufs=2) as rows,
        tc.tile_pool(name="psum", bufs=2, space="PSUM") as psum,
    ):
        # ---- compile-time constants (no DMA) ----
        onecol = consts.tile([1, 128], BF16, name="onecol")
        nc.vector.memset(onecol, 1.0)
        # scaled so that ot = oo * (1/sbc) = 2^OT_K * O / denom exactly
        ones128 = consts.tile([128, 128], BF16, name="ones128")
        nc.vector.memset(ones128, 2.0 ** (kv - OT_K))
        colones = consts.tile([128, 1], BF16, name="colones")
        nc.vector.memset(colones, 1.0)
        eps512 = consts.tile([1, 1], F32, name="eps512")
        nc.vector.memset(eps512, DIM * LN_EPS)

        # ---- DMA'd constants ----
        w8 = {}
        for nm, src in (("wq", wq), ("wk", wk), ("wv", wv), ("wo", wo)):
            t = consts.tile([128, 4, 512], F8, name=f"w_{nm}")
            nc.scalar.dma_start(out=t,
                                in_=src.rearrange("(kc p) f -> p kc f", p=128))
            w8[nm] = t
        wb = {}
        for nm, src in (("wc1", wc1), ("wc2", wc2)):
            t = consts.tile([128, 4, 512], BF16, name=f"w_{nm}")
            nc.scalar.dma_start(out=t,
                                in_=src.rearrange("(kc p) f -> p kc f", p=128))
            wb[nm] = t
        lngc = consts.tile([128, 4], F32, name="lngc")   # ln_g * sqrt(512)
        nc.scalar.dma_start(out=lngc, in_=lngd.rearrange("(c p) -> p c", p=128))
        lnbc = consts.tile([128, 4], F32, name="lnbc")
        nc.scalar.dma_start(out=lnbc, in_=lnbd.rearrange("(c p) -> p c", p=128))
        bc1c = consts.tile([128, 4], F32, name="bc1c")
        nc.scalar.dma_start(out=bc1c, in_=bc1d.rearrange("(c p) -> p c", p=128))
        bocbc = consts.tile([128, 4], F32, name="bocbc")
        nc.scalar.dma_start(out=bocbc, in_=bocbd.rearrange("(c p) -> p c", p=128))

        for sb in range(n_super):
            b0 = sb * SBATCH
            # ---- input DMAs (SP queue) ----
            xt8 = iop.tile([128, 4, SBATCH, 256], F8, name="xt8")
            ct8 = iop.tile([128, 4, SBATCH, 256], F8, name="ct8")
            ctb = iop.tile([128, 4, SBATCH, 256], BF16, name="ctb")
            for j in range(SBATCH):
                nc.sync.dma_start(
                    out=xt8[:, :, j, :],
                    in_=x8d[b0 + j].rearrange("(kc p) n -> p kc n", p=128))
                nc.sync.dma_start(
                    out=ct8[:, :, j, :],
                    in_=c8d[b0 + j].rearrange("(kc p) n -> p kc n", p=128))
                nc.sync.dma_start(
                    out=ctb[:, :, j, :],
                    in_=cbd[b0 + j].rearrange("(kc p) n -> p kc n", p=128))

            # ---- projections: fp8 DoubleRow ----
            qt = work.tile([128, 4, SBATCH, 256], BF16, name="qt")
            kt = work.tile([128, 4, SBATCH, 256], BF16, name="kt")
            for c in range(4):
                ps = psum.tile([128, NW], F32, tag="g", bufs=2)
                for i in range(2):
                    mm(ps, w8["wq"][:, 2 * i:2 * i + 2, c * 128:(c + 1) * 128],
                       xt8[:, 2 * i:2 * i + 2], start=i == 0, stop=i == 1,
                       perf_mode=DR)
                nc.scalar.copy(
                    out=qt[:, c].rearrange("p j n -> p (j n)"), in_=ps)
            for c in range(4):
                ps = psum.tile([128, NW], F32, tag="g", bufs=2)
                for i in range(2):
                    mm(ps, w8["wk"][:, 2 * i:2 * i + 2, c * 128:(c + 1) * 128],
                       ct8[:, 2 * i:2 * i + 2], start=i == 0, stop=i == 1,
                       perf_mode=DR)
                nc.scalar.copy(
                    out=kt[:, c].rearrange("p j n -> p (j n)"), in_=ps)
            # v token-major
            vt = work.tile([128, SBATCH, 2, 512], BF16, name="vt")
            for j in range(SBATCH):
                for mc in range(2):
                    ps = psum.tile([128, 512], F32, tag="g", bufs=2)
                    for i in range(2):
                        mm(ps, ct8[:, 2 * i:2 * i + 2, j, mc * 128:(mc + 1) * 128],
                           w8["wv"][:, 2 * i:2 * i + 2], start=i == 0, stop=i == 1,
                           perf_mode=DR)
                    nc.vector.tensor_copy(out=vt[:, j, mc, :], in_=ps)

            # ---- context branch: h = c @ Wc1 + bc1 (bf16, feature-major) ----
            ht = work.tile([128, 4, NW], BF16, name="ht")
            sqt = work.tile([128, 4, NW], BF16, name="sqt")
            for c in range(4):
                ps = psum.tile([128, NW], F32, tag="g", bufs=2)
                for kc in range(4):
                    mm(ps, wb["wc1"][:, kc, c * 128:(c + 1) * 128],
                       ctb[:, kc].rearrange("p j n -> p (j n)"),
                       start=kc == 0, stop=kc == 3)
                nc.scalar.activation(out=ht[:, c], in_=ps, func=AF.Identity,
                                     bias=bc1c[:, c:c + 1])
                nc.gpsimd.tensor_mul(out=sqt[:, c], in0=ht[:, c], in1=ht[:, c])

            # ---- LN stats ----
            mu_ps = psum.tile([128, NW], F32, tag="g", bufs=2)
            sq_ps = psum.tile([128, NW], F32, tag="g", bufs=2)
            for c in range(4):
                mm(mu_ps[0:1, :], colones, ht[:, c], start=c == 0, stop=c == 3)
            for c in range(4):
                mm(sq_ps[0:1, :], colones, sqt[:, c], start=c == 0, stop=c == 3)
            # 512*var = sq_sum - mu_sum^2/512
            mu_r = rows.tile([1, NW], F32, tag="r", bufs=6)
            nc.scalar.copy(out=mu_r, in_=mu_ps[0:1, :])
            ms_r = rows.tile([1, NW], F32, tag="r", bufs=6)
            nc.vector.scalar_tensor_tensor(
                out=ms_r, in0=mu_r, scalar=1.0 / DIM,
                in1=mu_r, op0=ALU.mult, op1=ALU.mult)
            var_r = rows.tile([1, NW], F32, tag="r", bufs=6)
            nc.vector.tensor_sub(out=var_r, in0=sq_ps[0:1, :], in1=ms_r)
            # a = rstd/sqrt(512) = exp(-0.5*ln(512*var + 512*eps))
            ln_r = rows.tile([1, NW], F32, tag="r", bufs=6)
            nc.scalar.activation(out=ln_r, in_=var_r, func=AF.Ln, bias=eps512)
            a_r = rows.tile([1, NW], BF16, tag="r", bufs=6)
            nc.scalar.activation(out=a_r, in_=ln_r, func=AF.Exp, scale=-0.5)
            # d = -(mu_sum/512) * a
            d_r = rows.tile([1, NW], BF16, tag="r", bufs=6)
            nc.vector.scalar_tensor_tensor(
                out=d_r, in0=mu_r, scalar=-1.0 / DIM,
                in1=a_r, op0=ALU.mult, op1=ALU.mult)
            # broadcast a,d to all 128 partitions (one 2-bank PSUM tile)
            ad_ps = psum.tile([128, 2, NW], F32, tag="s", bufs=2)
            mm(ad_ps[:, 0, :], onecol, a_r, start=True, stop=True)
            mm(ad_ps[:, 1, :], onecol, d_r, start=True, stop=True)
            ad_sb = work.tile([128, 2, NW], BF16, name="ad_sb")
            nc.vector.tensor_copy(out=ad_sb, in_=ad_ps)

            # ---- attention + interleaved LN-normalize / ctx2+out1 ----
            ot = work.tile([128, 4, SBATCH, 256], F8, name="ot")
            res = iop.tile([128, 4, SBATCH, 256], F32, name="res")

            def attn(c, j):
                s_ps = psum.tile([128, 2, NW], F32, tag="s", bufs=2)
                for h2 in range(2):
                    p0 = 64 * h2
                    for mc in range(2):
                        mm(s_ps[:, h2, mc * 256:(mc + 1) * 256],
                           kt[p0:p0 + 64, c, j, mc * 128:(mc + 1) * 128],
                           qt[p0:p0 + 64, c, j], start=True, stop=True)
                pt = ppool.tile([128, 2, 2, 256], BF16, tag="p", name="pt",
                                bufs=4)
                nc.scalar.activation(
                    out=pt.rearrange("p mc h2 n -> p h2 mc n"),
                    in_=s_ps.rearrange("p h2 (mc n) -> p h2 mc n", mc=2),
                    func=AF.Exp, scale=2.0 ** (-(kq + kk)))
                mbt = mbp.tile([128, 2, 2, 256], BF16, name="mbt")
                nc.sync.dma_start(
                    out=mbt,
                    in_=mbd[b0 + j, c].rearrange("mc h2 p n -> p mc h2 n"))
                nc.vector.tensor_mul(out=pt, in0=pt, in1=mbt)
                sbc = psum.tile([128, NW], F32, tag="g", bufs=2)
                for mc in range(2):
                    mm(sbc, ones128,
                       pt[:, mc].rearrange("p h n -> p (h n)"),
                       start=mc == 0, stop=mc == 1)
                oo = psum.tile([64, 2, 256], F32, tag="oo", bufs=2)
                for h2 in range(2):
                    hd = (2 * c + h2) * 64
                    for mc in range(2):
                        mm(oo[:, h2, :], vt[:, j, mc, hd:hd + 64],
                           pt[:, mc, h2, :], start=mc == 0, stop=mc == 1)
                rec_sb = ppool.tile([128, NW], F32, tag="rb", name="rec_sb",
                                    bufs=3)
                nc.vector.reciprocal(out=rec_sb, in_=sbc)
                for h2 in range(2):
                    nc.vector.tensor_mul(
                        out=ot[h2 * 64:(h2 + 1) * 64, c, j],
                        in0=oo[:, h2, :],
                        in1=rec_sb[h2 * 64:(h2 + 1) * 64,
                                   h2 * 256:(h2 + 1) * 256])

            def normalize(c):
                # rl = relu(((h*a + d)) * (g*sqrt(512)) + b), written in place
                nc.gpsimd.tensor_mul(out=ht[:, c], in0=ht[:, c],
                                     in1=ad_sb[:, 0, :])
                nc.gpsimd.tensor_add(out=ht[:, c], in0=ht[:, c],
                                     in1=ad_sb[:, 1, :])
                nc.scalar.activation(out=ht[:, c], in_=ht[:, c], func=AF.Relu,
                                     scale=lngc[:, c:c + 1],
                                     bias=lnbc[:, c:c + 1])  # Act (table: nlx)

            def ctx2wo(j):
                co = psum.tile([128, 2, NW], F32, tag="s", bufs=2)
                for f in range(4):
                    dst = co[:, f // 2, (f % 2) * 256:(f % 2) * 256 + 256]
                    for kc in range(4):
                        mm(dst, wb["wc2"][:, kc, f * 128:(f + 1) * 128],
                           ht[:, kc, j * 256:(j + 1) * 256],
                           start=kc == 0, stop=False)
                    for i in range(2):
                        mm(dst,
                           w8["wo"][:, 2 * i:2 * i + 2, f * 128:(f + 1) * 128],
                           ot[:, 2 * i:2 * i + 2, j, :],
                           start=False, stop=i == 1, perf_mode=DR)
                for f in range(4):
                    nc.scalar.activation(
                        out=res[:, f, j, :],
                        in_=co[:, f // 2, (f % 2) * 256:(f % 2) * 256 + 256],
                        func=AF.Identity, scale=2.0 ** (-g),
                        bias=bocbc[:, f:f + 1])
                nc.gpsimd.dma_start(
                    out=outT[b0 + j].rearrange("(c p) n -> p c n", p=128),
                    in_=res[:, :, j, :])

            for j in range(SBATCH):
                for c in range(4):
                    attn(c, j)
                    if j == 0:
                        normalize(c)
                ctx2wo(j)


def build(n_super, ks):
    nc = bacc.Bacc("TRN2", target_bir_lowering=False, debug=False,
                   num_devices=N_CORES)
    dt = nc.dram_tensor
    io = (
        dt("x8", [BPC, DIM, N], F8, kind="ExternalInput").ap(),
        dt("c8", [BPC, DIM, N], F8, kind="ExternalInput").ap(),
        dt("cb", [BPC, DIM, N], BF16, kind="ExternalInput").ap(),
        dt("mb", [BPC, 4, 2, 2, 128, N], BF16, kind="ExternalInput").ap(),
        dt("wq", [DIM, DIM], F8, kind="ExternalInput").ap(),
        dt("wk", [DIM, DIM], F8, kind="ExternalInput").ap(),
        dt("wv", [DIM, DIM], F8, kind="ExternalInput").ap(),
        dt("wo", [DIM, DIM], F8, kind="ExternalInput").ap(),
        dt("wc1", [DIM, DIM], BF16, kind="ExternalInput").ap(),
        dt("wc2", [DIM, DIM], BF16, kind="ExternalInput").ap(),
        dt("lng", [DIM], F32, kind="ExternalInput").ap(),
        dt("lnb", [DIM], F32, kind="ExternalInput").ap(),
        dt("bc1", [DIM], F32, kind="ExternalInput").ap(),
        dt("bocb", [DIM], F32, kind="ExternalInput").ap(),
        dt("outT", [BPC, DIM, N], F32, kind="ExternalOutput").ap(),
    )
    with tile.TileContext(nc) as tc:
        _emit(nc, tc, io, n_super, ks)
    nc.compile()
    return nc


def _k_of(absmax):
    return int(math.floor(math.log2(120.0 / max(absmax, 1e-30))))


def prep_in_maps(x, context, mask, Wq, Wk, Wv, Wc1, bc1, ln_g, ln_b, Wc2, bc2,
                 Wo, bo, bias_table, rel_index):
    f = np.float32
    x = np.asarray(x, f)
    context = np.asarray(context, f)
    mask = np.asarray(mask)
    Wq = np.asarray(Wq, f) * SCALE
    Wk = np.asarray(Wk, f)
    Wv = np.asarray(Wv, f)
    Wo = np.asarray(Wo, f)
    Wc1 = np.asarray(Wc1, f)
    Wc2 = np.asarray(Wc2, f)

    kq = _k_of(np.abs(Wq).max())
    kk = _k_of(np.abs(Wk).max())
    kv = _k_of(np.abs(Wv).max())
    kwo = _k_of(np.abs(Wo).max())
    ks = (kq, kk, kv, kwo)
    g = OT_K + kwo

    xT = np.ascontiguousarray(
        x.reshape(N_CORES, BPC, N, DIM).transpose(0, 1, 3, 2))
    cT = np.ascontiguousarray(
        context.reshape(N_CORES, BPC, N, DIM).transpose(0, 1, 3, 2))
    x8 = xT.astype(NP8)
    c8 = cT.astype(NP8)
    cb = cT.astype(NPBF)

    # mb[core, b, c, mc, h2, p, n] = maskT[b, m, n] * exp(bias)[h, m, n]
    expBT = np.exp(
        np.asarray(bias_table, f)[np.asarray(rel_index)].transpose(2, 1, 0))
    # expBT: [H, m, n]; maskT: [core, b, m, n]
    mT = mask.reshape(N_CORES, BPC, N, N).transpose(0, 1, 3, 2).astype(f)
    mbf = mT[:, :, None, :, :] * expBT[None, None, :, :, :]  # [cr,b,h,m,n]
    mbf = mbf.reshape(N_CORES, BPC, 4, 2, 2, 128, N).transpose(
        0, 1, 2, 4, 3, 5, 6)  # [cr, b, c, mc, h2, p, n]
    mb = np.ascontiguousarray(mbf).astype(NPBF)

    shared = dict(
        wq=np.ascontiguousarray(Wq * 2.0 ** kq).astype(NP8),
        wk=np.ascontiguousarray(Wk * 2.0 ** kk).astype(NP8),
        wv=np.ascontiguousarray(Wv * 2.0 ** kv).astype(NP8),
        wo=np.ascontiguousarray(Wo * 2.0 ** kwo).astype(NP8),
        wc1=np.ascontiguousarray(Wc1).astype(NPBF),
        wc2=np.ascontiguousarray(Wc2 * 2.0 ** g).astype(NPBF),
        lng=np.ascontiguousarray(np.asarray(ln_g, f) * math.sqrt(DIM)),
        lnb=np.ascontiguousarray(np.asarray(ln_b, f)),
        bc1=np.ascontiguousarray(np.asarray(bc1, f)),
        bocb=np.ascontiguousarray(np.asarray(bo, f) + np.asarray(bc2, f)),
    )
    in_maps = [dict(x8=x8[c], c8=c8[c], cb=cb[c], mb=mb[c], **shared)
               for c in range(N_CORES)]
    return in_maps, ks


_nc_cache = {}


def _get_nc(n_super, ks):
    key = (n_super, ks)
    if key not in _nc_cache:
        _nc_cache[key] = build(n_super, ks)
    return _nc_cache[key]


def assemble_out(results):
    outT = np.stack([results[c]["outT"] for c in range(N_CORES)])
    return np.ascontiguousarray(
        outT.transpose(0, 1, 3, 2).reshape(B, N, DIM)).astype(np.float32)


def kernel(**inputs):
    in_maps, ks = prep_in_maps(**inputs)
    nc = _get_nc(NSUPER, ks)
    res = run_bass_kernel_spmd(nc, in_maps, core_ids=list(range(N_CORES)))
    return assemble_out(res.results)


# revision 17
# speedup vs baseline: 1.0904x; 1.0904x over previous
"""ContextAwareAttention Trainium2 kernel (v2).

Strategy (sized for the TimelineSim cost model):
  - Data-parallel over batch: B=128 -> 16 batches/core x 8 cores; SBATCH=2
    batches per superbatch ("SB") iteration.
  - fp8e4m3 DoubleRow matmuls (0.5 cyc/row, 2x contraction per instruction)
    for the q/k/v projections and Wo: 4x fewer PE cycles than fp32r.
    Weights are pre-scaled by powers of two into fp8 range; scales cancel
    exactly (exp scale / V-ones column value / output copy scale).
  - bf16 on the element-wise engines (DVE 2x perf modes).
  - Softmax denominator rides the attention@V matmul as a 65th V column;
    reciprocal on a [1,512] row; broadcast back by one PE matmul.
  - mask and exp(rel-pos-bias) are pre-multiplied on the host into one bf16
    [b, head-pair, m, n] table -> single fused P multiply per (c,j).
  - LayerNorm rstd = exp(-0.5*ln(512*var+512*eps)): Ln/Exp/Relu/Square share
    one activation table set -> zero act-table reloads.
  - ctx2 (Wc2) and out1 (Wo) accumulate into one PSUM tile per (chunk, j);
    one copy applies the 2^-g rescale plus the combined bias.
"""

import math

import numpy as np
import ml_dtypes

import concourse.bass as bass  # noqa: F401
import concourse.mybir as mybir
import concourse.tile as tile
from concourse import bacc
from concourse.bass_utils import run_bass_kernel_spmd

B, N, DIM, H, D = 128, 256, 512, 8, 64
N_CORES = 8
BPC = B // N_CORES            # 16
SBATCH = 2
NSUPER = BPC // SBATCH        # 8
SCALE = D ** -0.5
LN_EPS = 1e-5
F32 = mybir.dt.float32
F32R = mybir.dt.float32r
BF16 = mybir.dt.bfloat16
F8 = mybir.dt.float8e4
NW = SBATCH * N               # 512
NP8 = np.dtype(ml_dtypes.float8_e4m3)
NPBF = np.dtype(ml_dtypes.bfloat16)

AF = mybir.ActivationFunctionType
ALU = mybir.AluOpType
DR = mybir.MatmulPerfMode.DoubleRow

OT_K = 5   # ot is stored as 2^OT_K * O/denom (fp8 range health)


def _emit(nc, tc, io, n_super, ks):
    (x8d, c8d, cbd, mbd, wq, wk, wv, wo, wc1, wc2, lngd, lnbd,
     bc1d, bocbd, outT) = io
    kq, kk, kv, kwo = ks
    g = OT_K + kwo

    def mm(out, lhsT, rhs, start, stop, perf_mode=None):
        nc.tensor.matmul(out, lhsT, rhs, start=start, stop=stop,
                         perf_mode=perf_mode)

    with (
        nc.allow_low_precision(reason="fp8/bf16 design, verified vs oracle"),
        tc.tile_pool(name="consts", bufs=1) as consts,
        tc.tile_pool(name="io", bufs=3) as iop,
        tc.tile_pool(name="mb", bufs=4) as mbp,
        tc.tile_pool(name="work", bufs=1) as work,
        tc.tile_pool(name="pp", bufs=3) as ppool,
        tc.tile_pool(name="rows", bufs=2) as rows,
        tc.tile_pool(name="psum", bufs=2, space="PSUM") as psum,
    ):
        # ---- compile-time constants (no DMA) ----
        onecol = consts.tile([1, 128], BF16, name="onecol")
        nc.vector.memset(onecol, 1.0)
        # scaled so that ot = oo * (1/sbc) = 2^OT_K * O / denom exactly
        ones128 = consts.tile([128, 128], BF16, name="ones128")
        nc.vector.memset(ones128, 2.0 ** (kv - OT_K))
        colones = consts.tile([128, 1], BF16, name="colones")
        nc.vector.memset(colones, 1.0)
        eps512 = consts.tile([1, 1], F32, name="eps512")
        nc.vector.memset(eps512, DIM * LN_EPS)

        # ---- DMA'd constants ----
        w8 = {}
        for nm, src in (("wq", wq), ("wk", wk), ("wv", wv), ("wo", wo)):
            t = consts.tile([128, 4, 512], F8, name=f"w_{nm}")
            nc.scalar.dma_start(out=t,
                                in_=src.rearrange("(kc p) f -> p kc f", p=128))
            w8[nm] = t
        wb = {}
        for nm, src in (("wc1", wc1), ("wc2", wc2)):
            t = consts.tile([128, 4, 512], BF16, name=f"w_{nm}")
            nc.scalar.dma_start(out=t,
                                in_=src.rearrange("(kc p) f -> p kc f", p=128))
            wb[nm] = t
        lngc = consts.tile([128, 4], F32, name="lngc")   # ln_g * sqrt(512)
        nc.scalar.dma_start(out=lngc, in_=lngd.rearrange("(c p) -> p c", p=128))
        lnbc = consts.tile([128, 4], F32, name="lnbc")
        nc.scalar.dma_start(out=lnbc, in_=lnbd.rearrange("(c p) -> p c", p=128))
        bc1c = consts.tile([128, 4], F32, name="bc1c")
        nc.scalar.dma_start(out=bc1c, in_=bc1d.rearrange("(c p) -> p c", p=128))
        bocbc = consts.tile([128, 4], F32, name="bocbc")
        nc.scalar.dma_start(out=bocbc, in_=bocbd.rearrange("(c p) -> p c", p=128))

        for sb in range(n_super):
            b0 = sb * SBATCH
            # ---- input DMAs (SP queue) ----
            xt8 = iop.tile([128, 4, SBATCH, 256], F8, name="xt8")
            ct8 = iop.tile([128, 4, SBATCH, 256], F8, name="ct8")
            ctb = iop.tile([128, 4, SBATCH, 256], BF16, name="ctb")
            for j in range(SBATCH):
                nc.sync.dma_start(
                    out=xt8[:, :, j, :],
                    in_=x8d[b0 + j].rearrange("(kc p) n -> p kc n", p=128))
                nc.sync.dma_start(
                    out=ct8[:, :, j, :],
                    in_=c8d[b0 + j].rearrange("(kc p) n -> p kc n", p=128))
                nc.sync.dma_start(
                    out=ctb[:, :, j, :],
                    in_=cbd[b0 + j].rearrange("(kc p) n -> p kc n", p=128))

            # ---- projections: fp8 DoubleRow ----
            qt = work.tile([128, 4, SBATCH, 256], BF16, name="qt")
            kt = work.tile([128, 4, SBATCH, 256], BF16, name="kt")
            for c in range(4):
                ps = psum.tile([128, NW], F32, tag="g", bufs=2)
                for i in range(2):
                    mm(ps, w8["wq"][:, 2 * i:2 * i + 2, c * 128:(c + 1) * 128],
                       xt8[:, 2 * i:2 * i + 2], start=i == 0, stop=i == 1,
                       perf_mode=DR)
                nc.scalar.copy(
                    out=qt[:, c].rearrange("p j n -> p (j n)"), in_=ps)
            for c in range(4):
                ps = psum.tile([128, NW], F32, tag="g", bufs=2)
                for i in range(2):
                    mm(ps, w8["wk"][:, 2 * i:2 * i + 2, c * 128:(c + 1) * 128],
                       ct8[:, 2 * i:2 * i + 2], start=i == 0, stop=i == 1,
                       perf_mode=DR)
                nc.scalar.copy(
                    out=kt[:, c].rearrange("p j n -> p (j n)"), in_=ps)
            # v token-major
            vt = work.tile([128, SBATCH, 2, 512], BF16, name="vt")
            for j in range(SBATCH):
                for mc in range(2):
                    ps = psum.tile([128, 512], F32, tag="g", bufs=2)
                    for i in range(2):
                        mm(ps, ct8[:, 2 * i:2 * i + 2, j, mc * 128:(mc + 1) * 128],
                           w8["wv"][:, 2 * i:2 * i + 2], start=i == 0, stop=i == 1,
                           perf_mode=DR)
                    nc.vector.tensor_copy(out=vt[:, j, mc, :], in_=ps)

            # ---- context branch: h = c @ Wc1 + bc1 (bf16, feature-major) ----
            ht = work.tile([128, 4, NW], BF16, name="ht")
            sqt = work.tile([128, 4, NW], BF16, name="sqt")
            for c in range(4):
                ps = psum.tile([128, NW], F32, tag="g", bufs=2)
                for kc in range(4):
                    mm(ps, wb["wc1"][:, kc, c * 128:(c + 1) * 128],
                       ctb[:, kc].rearrange("p j n -> p (j n)"),
                       start=kc == 0, stop=kc == 3)
                nc.scalar.activation(out=ht[:, c], in_=ps, func=AF.Identity,
                                     bias=bc1c[:, c:c + 1])
                nc.gpsimd.tensor_mul(out=sqt[:, c], in0=ht[:, c], in1=ht[:, c])

            # ---- LN stats (emitted as closures, interleaved into the
            # attention stream to avoid head-of-line blocking) ----
            state = {}

            def ln_stats_a():
                mu_ps = psum.tile([128, NW], F32, tag="g", bufs=2)
                sq_ps = psum.tile([128, NW], F32, tag="g", bufs=2)
                for c in range(4):
                    mm(mu_ps[0:1, :], colones, ht[:, c],
                       start=c == 0, stop=c == 3)
                for c in range(4):
                    mm(sq_ps[0:1, :], colones, sqt[:, c],
                       start=c == 0, stop=c == 3)
                mu_r = rows.tile([1, NW], F32, tag="r", bufs=6)
                nc.scalar.copy(out=mu_r, in_=mu_ps[0:1, :])
                ms_r = rows.tile([1, NW], F32, tag="r", bufs=6)
                nc.vector.scalar_tensor_tensor(
                    out=ms_r, in0=mu_r, scalar=1.0 / DIM,
                    in1=mu_r, op0=ALU.mult, op1=ALU.mult)
                var_r = rows.tile([1, NW], F32, tag="r", bufs=6)
                nc.vector.tensor_sub(out=var_r, in0=sq_ps[0:1, :], in1=ms_r)
                state.update(mu_r=mu_r, var_r=var_r)

            def ln_stats_b():
                mu_r, var_r = state["mu_r"], state["var_r"]
                ln_r = rows.tile([1, NW], F32, tag="r", bufs=6)
                nc.scalar.activation(out=ln_r, in_=var_r, func=AF.Ln,
                                     bias=eps512)
                a_r = rows.tile([1, NW], BF16, tag="r", bufs=6)
                nc.scalar.activation(out=a_r, in_=ln_r, func=AF.Exp,
                                     scale=-0.5)
                d_r = rows.tile([1, NW], BF16, tag="r", bufs=6)
                nc.vector.scalar_tensor_tensor(
                    out=d_r, in0=mu_r, scalar=-1.0 / DIM,
                    in1=a_r, op0=ALU.mult, op1=ALU.mult)
                ad_ps = psum.tile([128, 2, NW], F32, tag="s", bufs=2)
                mm(ad_ps[:, 0, :], onecol, a_r, start=True, stop=True)
                mm(ad_ps[:, 1, :], onecol, d_r, start=True, stop=True)
                ad_sb = work.tile([128, 2, NW], BF16, name="ad_sb")
                nc.vector.tensor_copy(out=ad_sb, in_=ad_ps)
                state["ad_sb"] = ad_sb

            # ---- attention + interleaved LN-normalize / ctx2+out1 ----
            ot = work.tile([128, 4, SBATCH, 256], F8, name="ot")
            res = iop.tile([128, 4, SBATCH, 256], F32, name="res")

            def attn(c, j):
                s_ps = psum.tile([128, 2, NW], F32, tag="s", bufs=2)
                for h2 in range(2):
                    p0 = 64 * h2
                    for mc in range(2):
                        mm(s_ps[:, h2, mc * 256:(mc + 1) * 256],
                           kt[p0:p0 + 64, c, j, mc * 128:(mc + 1) * 128],
                           qt[p0:p0 + 64, c, j], start=True, stop=True)
                pt = ppool.tile([128, 2, 2, 256], BF16, tag="p", name="pt",
                                bufs=4)
                nc.scalar.activation(
                    out=pt.rearrange("p mc h2 n -> p h2 mc n"),
                    in_=s_ps.rearrange("p h2 (mc n) -> p h2 mc n", mc=2),
                    func=AF.Exp, scale=2.0 ** (-(kq + kk)))
                mbt = mbp.tile([128, 2, 2, 256], BF16, name="mbt")
                nc.sync.dma_start(
                    out=mbt,
                    in_=mbd[b0 + j, c].rearrange("mc h2 p n -> p mc h2 n"))
                nc.vector.tensor_mul(out=pt, in0=pt, in1=mbt)
                sbc = psum.tile([128, NW], F32, tag="g", bufs=2)
                for mc in range(2):
                    mm(sbc, ones128,
                       pt[:, mc].rearrange("p h n -> p (h n)"),
                       start=mc == 0, stop=mc == 1)
                oo = psum.tile([64, 2, 256], F32, tag="oo", bufs=2)
                for h2 in range(2):
                    hd = (2 * c + h2) * 64
                    for mc in range(2):
                        mm(oo[:, h2, :], vt[:, j, mc, hd:hd + 64],
                           pt[:, mc, h2, :], start=mc == 0, stop=mc == 1)
                rec_sb = ppool.tile([128, NW], F32, tag="rb", name="rec_sb",
                                    bufs=3)
                nc.vector.reciprocal(out=rec_sb, in_=sbc)
                for h2 in range(2):
                    nc.vector.tensor_mul(
                        out=ot[h2 * 64:(h2 + 1) * 64, c, j],
                        in0=oo[:, h2, :],
                        in1=rec_sb[h2 * 64:(h2 + 1) * 64,
                                   h2 * 256:(h2 + 1) * 256])

            def normalize(c):
                # rl = relu(((h*a + d)) * (g*sqrt(512)) + b), written in place
                ad_sb = state["ad_sb"]
                nc.gpsimd.tensor_mul(out=ht[:, c], in0=ht[:, c],
                                     in1=ad_sb[:, 0, :])
                nc.gpsimd.tensor_add(out=ht[:, c], in0=ht[:, c],
                                     in1=ad_sb[:, 1, :])
                nc.scalar.activation(out=ht[:, c], in_=ht[:, c], func=AF.Relu,
                                     scale=lngc[:, c:c + 1],
                                     bias=lnbc[:, c:c + 1])  # Act (table: nlx)

            def ctx2wo(j):
                co = psum.tile([128, 2, NW], F32, tag="s", bufs=2)
                for f in range(4):
                    dst = co[:, f // 2, (f % 2) * 256:(f % 2) * 256 + 256]
                    for kc in range(4):
                        mm(dst, wb["wc2"][:, kc, f * 128:(f + 1) * 128],
                           ht[:, kc, j * 256:(j + 1) * 256],
                           start=kc == 0, stop=False)
                    for i in range(2):
                        mm(dst,
                           w8["wo"][:, 2 * i:2 * i + 2, f * 128:(f + 1) * 128],
                           ot[:, 2 * i:2 * i + 2, j, :],
                           start=False, stop=i == 1, perf_mode=DR)
                for f in range(4):
                    nc.scalar.activation(
                        out=res[:, f, j, :],
                        in_=co[:, f // 2, (f % 2) * 256:(f % 2) * 256 + 256],
                        func=AF.Identity, scale=2.0 ** (-g),
                        bias=bocbc[:, f:f + 1])
                nc.gpsimd.dma_start(
                    out=outT[b0 + j].rearrange("(c p) n -> p c n", p=128),
                    in_=res[:, :, j, :])

            attn(0, 0)
            ln_stats_a()
            attn(1, 0)
            ln_stats_b()
            attn(2, 0)
            normalize(0)
            normalize(1)
            attn(3, 0)
            normalize(2)
            normalize(3)
            ctx2wo(0)
            for c in range(4):
                attn(c, 1)
            ctx2wo(1)


def build(n_super, ks):
    # Pin the activation table: expose only natural_log_exp_and_others
    # (contains Exp/Ln/Relu/Identity/Copy/Square) to the act-table-load
    # insertion pass so it never flip-flops between sets.  Other entries are
    # emptied (not removed) to keep act_func_set_id indices valid.
    import concourse.bacc as bacc_mod
    from concourse.hw_specs import get_activation_tables as _gat
    def pinned_tables(arch):
        tabs = _gat(arch)
        return {name: (s if name == "natural_log_exp_and_others" else set())
                for name, s in tabs.items()}

    nc = bacc.Bacc("TRN2", target_bir_lowering=False, debug=False,
                   num_devices=N_CORES)
    dt = nc.dram_tensor
    io = (
        dt("x8", [BPC, DIM, N], F8, kind="ExternalInput").ap(),
        dt("c8", [BPC, DIM, N], F8, kind="ExternalInput").ap(),
        dt("cb", [BPC, DIM, N], BF16, kind="ExternalInput").ap(),
        dt("mb", [BPC, 4, 2, 2, 128, N], BF16, kind="ExternalInput").ap(),
        dt("wq", [DIM, DIM], F8, kind="ExternalInput").ap(),
        dt("wk", [DIM, DIM], F8, kind="ExternalInput").ap(),
        dt("wv", [DIM, DIM], F8, kind="ExternalInput").ap(),
        dt("wo", [DIM, DIM], F8, kind="ExternalInput").ap(),
        dt("wc1", [DIM, DIM], BF16, kind="ExternalInput").ap(),
        dt("wc2", [DIM, DIM], BF16, kind="ExternalInput").ap(),
        dt("lng", [DIM], F32, kind="ExternalInput").ap(),
        dt("lnb", [DIM], F32, kind="ExternalInput").ap(),
        dt("bc1", [DIM], F32, kind="ExternalInput").ap(),
        dt("bocb", [DIM], F32, kind="ExternalInput").ap(),
        dt("outT", [BPC, DIM, N], F32, kind="ExternalOutput").ap(),
    )
    with tile.TileContext(nc) as tc:
        _emit(nc, tc, io, n_super, ks)
    saved = bacc_mod.get_activation_tables
    bacc_mod.get_activation_tables = pinned_tables
    try:
        nc.compile()
    finally:
        bacc_mod.get_activation_tables = saved
    return nc


def _k_of(absmax):
    return int(math.floor(math.log2(120.0 / max(absmax, 1e-30))))


def prep_in_maps(x, context, mask, Wq, Wk, Wv, Wc1, bc1, ln_g, ln_b, Wc2, bc2,
                 Wo, bo, bias_table, rel_index):
    f = np.float32
    x = np.asarray(x, f)
    context = np.asarray(context, f)
    mask = np.asarray(mask)
    Wq = np.asarray(Wq, f) * SCALE
    Wk = np.asarray(Wk, f)
    Wv = np.asarray(Wv, f)
    Wo = np.asarray(Wo, f)
    Wc1 = np.asarray(Wc1, f)
    Wc2 = np.asarray(Wc2, f)

    kq = _k_of(np.abs(Wq).max())
    kk = _k_of(np.abs(Wk).max())
    kv = _k_of(np.abs(Wv).max())
    kwo = _k_of(np.abs(Wo).max())
    ks = (kq, kk, kv, kwo)
    g = OT_K + kwo

    xT = np.ascontiguousarray(
        x.reshape(N_CORES, BPC, N, DIM).transpose(0, 1, 3, 2))
    cT = np.ascontiguousarray(
        context.reshape(N_CORES, BPC, N, DIM).transpose(0, 1, 3, 2))
    x8 = xT.astype(NP8)
    c8 = cT.astype(NP8)
    cb = cT.astype(NPBF)

    # mb[core, b, c, mc, h2, p, n] = maskT[b, m, n] * exp(bias)[h, m, n]
    expBT = np.exp(
        np.asarray(bias_table, f)[np.asarray(rel_index)].transpose(2, 1, 0))
    # expBT: [H, m, n]; maskT: [core, b, m, n]
    mT = mask.reshape(N_CORES, BPC, N, N).transpose(0, 1, 3, 2).astype(f)
    mbf = mT[:, :, None, :, :] * expBT[None, None, :, :, :]  # [cr,b,h,m,n]
    mbf = mbf.reshape(N_CORES, BPC, 4, 2, 2, 128, N).transpose(
        0, 1, 2, 4, 3, 5, 6)  # [cr, b, c, mc, h2, p, n]
    mb = np.ascontiguousarray(mbf).astype(NPBF)

    shared = dict(
        wq=np.ascontiguousarray(Wq * 2.0 ** kq).astype(NP8),
        wk=np.ascontiguousarray(Wk * 2.0 ** kk).astype(NP8),
        wv=np.ascontiguousarray(Wv * 2.0 ** kv).astype(NP8),
        wo=np.ascontiguousarray(Wo * 2.0 ** kwo).astype(NP8),
        wc1=np.ascontiguousarray(Wc1).astype(NPBF),
        wc2=np.ascontiguousarray(Wc2 * 2.0 ** g).astype(NPBF),
        lng=np.ascontiguousarray(np.asarray(ln_g, f) * math.sqrt(DIM)),
        lnb=np.ascontiguousarray(np.asarray(ln_b, f)),
        bc1=np.ascontiguousarray(np.asarray(bc1, f)),
        bocb=np.ascontiguousarray(np.asarray(bo, f) + np.asarray(bc2, f)),
    )
    in_maps = [dict(x8=x8[c], c8=c8[c], cb=cb[c], mb=mb[c], **shared)
               for c in range(N_CORES)]
    return in_maps, ks


_nc_cache = {}


def _get_nc(n_super, ks):
    key = (n_super, ks)
    if key not in _nc_cache:
        _nc_cache[key] = build(n_super, ks)
    return _nc_cache[key]


def assemble_out(results):
    outT = np.stack([results[c]["outT"] for c in range(N_CORES)])
    return np.ascontiguousarray(
        outT.transpose(0, 1, 3, 2).reshape(B, N, DIM)).astype(np.float32)


def kernel(**inputs):
    in_maps, ks = prep_in_maps(**inputs)
    nc = _get_nc(NSUPER, ks)
    res = run_bass_kernel_spmd(nc, in_maps, core_ids=list(range(N_CORES)))
    return assemble_out(res.results)


# revision 18
# speedup vs baseline: 1.0964x; 1.0055x over previous
"""ContextAwareAttention Trainium2 kernel (v2).

Strategy (sized for the TimelineSim cost model):
  - Data-parallel over batch: B=128 -> 16 batches/core x 8 cores; SBATCH=2
    batches per superbatch ("SB") iteration.
  - fp8e4m3 DoubleRow matmuls (0.5 cyc/row, 2x contraction per instruction)
    for the q/k/v projections and Wo: 4x fewer PE cycles than fp32r.
    Weights are pre-scaled by powers of two into fp8 range; scales cancel
    exactly (exp scale / V-ones column value / output copy scale).
  - bf16 on the element-wise engines (DVE 2x perf modes).
  - Softmax denominator rides the attention@V matmul as a 65th V column;
    reciprocal on a [1,512] row; broadcast back by one PE matmul.
  - mask and exp(rel-pos-bias) are pre-multiplied on the host into one bf16
    [b, head-pair, m, n] table -> single fused P multiply per (c,j).
  - LayerNorm rstd = exp(-0.5*ln(512*var+512*eps)): Ln/Exp/Relu/Square share
    one activation table set -> zero act-table reloads.
  - ctx2 (Wc2) and out1 (Wo) accumulate into one PSUM tile per (chunk, j);
    one copy applies the 2^-g rescale plus the combined bias.
"""

import math

import numpy as np
import ml_dtypes

import concourse.bass as bass  # noqa: F401
import concourse.mybir as mybir
import concourse.tile as tile
from concourse import bacc
from concourse.bass_utils import run_bass_kernel_spmd

B, N, DIM, H, D = 128, 256, 512, 8, 64
N_CORES = 8
BPC = B // N_CORES            # 16
SBATCH = 2
NSUPER = BPC // SBATCH        # 8
SCALE = D ** -0.5
LN_EPS = 1e-5
F32 = mybir.dt.float32
F32R = mybir.dt.float32r
BF16 = mybir.dt.bfloat16
F8 = mybir.dt.float8e4
NW = SBATCH * N               # 512
NP8 = np.dtype(ml_dtypes.float8_e4m3)
NPBF = np.dtype(ml_dtypes.bfloat16)

AF = mybir.ActivationFunctionType
ALU = mybir.AluOpType
DR = mybir.MatmulPerfMode.DoubleRow

OT_K = 5   # ot is stored as 2^OT_K * O/denom (fp8 range health)


def _emit(nc, tc, io, n_super, ks):
    (x8d, c8d, cbd, mbd, wq, wk, wv, wo, wc1, wc2, lngd, lnbd,
     bc1d, bocbd, outT) = io
    kq, kk, kv, kwo = ks
    g = OT_K + kwo

    def mm(out, lhsT, rhs, start, stop, perf_mode=None):
        nc.tensor.matmul(out, lhsT, rhs, start=start, stop=stop,
                         perf_mode=perf_mode)

    with (
        nc.allow_low_precision(reason="fp8/bf16 design, verified vs oracle"),
        tc.tile_pool(name="consts", bufs=1) as consts,
        tc.tile_pool(name="io", bufs=3) as iop,
        tc.tile_pool(name="mb", bufs=6) as mbp,
        tc.tile_pool(name="work", bufs=1) as work,
        tc.tile_pool(name="pp", bufs=3) as ppool,
        tc.tile_pool(name="rows", bufs=2) as rows,
        tc.tile_pool(name="psum", bufs=2, space="PSUM") as psum,
    ):
        # ---- compile-time constants (no DMA) ----
        onecol = consts.tile([1, 128], BF16, name="onecol")
        nc.vector.memset(onecol, 1.0)
        # scaled so that ot = oo * (1/sbc) = 2^OT_K * O / denom exactly
        ones128 = consts.tile([128, 128], BF16, name="ones128")
        nc.vector.memset(ones128, 2.0 ** (kv - OT_K))
        colones = consts.tile([128, 1], BF16, name="colones")
        nc.vector.memset(colones, 1.0)
        eps512 = consts.tile([1, 1], F32, name="eps512")
        nc.vector.memset(eps512, DIM * LN_EPS)

        # ---- DMA'd constants ----
        w8 = {}
        for nm, src in (("wq", wq), ("wk", wk), ("wv", wv), ("wo", wo)):
            t = consts.tile([128, 4, 512], F8, name=f"w_{nm}")
            nc.scalar.dma_start(out=t,
                                in_=src.rearrange("(kc p) f -> p kc f", p=128))
            w8[nm] = t
        wb = {}
        for nm, src in (("wc1", wc1), ("wc2", wc2)):
            t = consts.tile([128, 4, 512], BF16, name=f"w_{nm}")
            nc.scalar.dma_start(out=t,
                                in_=src.rearrange("(kc p) f -> p kc f", p=128))
            wb[nm] = t
        lngc = consts.tile([128, 4], F32, name="lngc")   # ln_g * sqrt(512)
        nc.scalar.dma_start(out=lngc, in_=lngd.rearrange("(c p) -> p c", p=128))
        lnbc = consts.tile([128, 4], F32, name="lnbc")
        nc.scalar.dma_start(out=lnbc, in_=lnbd.rearrange("(c p) -> p c", p=128))
        bc1c = consts.tile([128, 4], F32, name="bc1c")
        nc.scalar.dma_start(out=bc1c, in_=bc1d.rearrange("(c p) -> p c", p=128))
        bocbc = consts.tile([128, 4], F32, name="bocbc")
        nc.scalar.dma_start(out=bocbc, in_=bocbd.rearrange("(c p) -> p c", p=128))

        for sb in range(n_super):
            b0 = sb * SBATCH
            # ---- input DMAs (SP queue) ----
            xt8 = iop.tile([128, 4, SBATCH, 256], F8, name="xt8")
            ct8 = iop.tile([128, 4, SBATCH, 256], F8, name="ct8")
            ctb = iop.tile([128, 4, SBATCH, 256], BF16, name="ctb")
            for j in range(SBATCH):
                nc.sync.dma_start(
                    out=xt8[:, :, j, :],
                    in_=x8d[b0 + j].rearrange("(kc p) n -> p kc n", p=128))
                nc.sync.dma_start(
                    out=ct8[:, :, j, :],
                    in_=c8d[b0 + j].rearrange("(kc p) n -> p kc n", p=128))
                nc.sync.dma_start(
                    out=ctb[:, :, j, :],
                    in_=cbd[b0 + j].rearrange("(kc p) n -> p kc n", p=128))

            # ---- projections: fp8 DoubleRow ----
            qt = work.tile([128, 4, SBATCH, 256], BF16, name="qt")
            kt = work.tile([128, 4, SBATCH, 256], BF16, name="kt")
            for c in range(4):
                ps = psum.tile([128, NW], F32, tag="g", bufs=2)
                for i in range(2):
                    mm(ps, w8["wq"][:, 2 * i:2 * i + 2, c * 128:(c + 1) * 128],
                       xt8[:, 2 * i:2 * i + 2], start=i == 0, stop=i == 1,
                       perf_mode=DR)
                nc.scalar.copy(
                    out=qt[:, c].rearrange("p j n -> p (j n)"), in_=ps)
            for c in range(4):
                ps = psum.tile([128, NW], F32, tag="g", bufs=2)
                for i in range(2):
                    mm(ps, w8["wk"][:, 2 * i:2 * i + 2, c * 128:(c + 1) * 128],
                       ct8[:, 2 * i:2 * i + 2], start=i == 0, stop=i == 1,
                       perf_mode=DR)
                nc.scalar.copy(
                    out=kt[:, c].rearrange("p j n -> p (j n)"), in_=ps)
            # v token-major
            vt = work.tile([128, SBATCH, 2, 512], BF16, name="vt")
            for j in range(SBATCH):
                for mc in range(2):
                    ps = psum.tile([128, 512], F32, tag="g", bufs=2)
                    for i in range(2):
                        mm(ps, ct8[:, 2 * i:2 * i + 2, j, mc * 128:(mc + 1) * 128],
                           w8["wv"][:, 2 * i:2 * i + 2], start=i == 0, stop=i == 1,
                           perf_mode=DR)
                    nc.vector.tensor_copy(out=vt[:, j, mc, :], in_=ps)

            # ---- context branch: h = c @ Wc1 + bc1 (bf16, feature-major) ----
            ht = work.tile([128, 4, NW], BF16, name="ht")
            sqt = work.tile([128, 4, NW], BF16, name="sqt")
            for c in range(4):
                ps = psum.tile([128, NW], F32, tag="g", bufs=2)
                for kc in range(4):
                    mm(ps, wb["wc1"][:, kc, c * 128:(c + 1) * 128],
                       ctb[:, kc].rearrange("p j n -> p (j n)"),
                       start=kc == 0, stop=kc == 3)
                nc.scalar.activation(out=ht[:, c], in_=ps, func=AF.Identity,
                                     bias=bc1c[:, c:c + 1])
                nc.gpsimd.tensor_mul(out=sqt[:, c], in0=ht[:, c], in1=ht[:, c])

            # ---- LN stats (emitted as closures, interleaved into the
            # attention stream to avoid head-of-line blocking) ----
            state = {}

            def ln_stats_a():
                mu_ps = psum.tile([128, NW], F32, tag="g", bufs=2)
                sq_ps = psum.tile([128, NW], F32, tag="g", bufs=2)
                for c in range(4):
                    mm(mu_ps[0:1, :], colones, ht[:, c],
                       start=c == 0, stop=c == 3)
                for c in range(4):
                    mm(sq_ps[0:1, :], colones, sqt[:, c],
                       start=c == 0, stop=c == 3)
                mu_r = rows.tile([1, NW], F32, tag="r", bufs=6)
                nc.scalar.copy(out=mu_r, in_=mu_ps[0:1, :])
                ms_r = rows.tile([1, NW], F32, tag="r", bufs=6)
                nc.vector.scalar_tensor_tensor(
                    out=ms_r, in0=mu_r, scalar=1.0 / DIM,
                    in1=mu_r, op0=ALU.mult, op1=ALU.mult)
                var_r = rows.tile([1, NW], F32, tag="r", bufs=6)
                nc.vector.tensor_sub(out=var_r, in0=sq_ps[0:1, :], in1=ms_r)
                state.update(mu_r=mu_r, var_r=var_r)

            def ln_stats_b():
                mu_r, var_r = state["mu_r"], state["var_r"]
                ln_r = rows.tile([1, NW], F32, tag="r", bufs=6)
                nc.scalar.activation(out=ln_r, in_=var_r, func=AF.Ln,
                                     bias=eps512)
                a_r = rows.tile([1, NW], BF16, tag="r", bufs=6)
                nc.scalar.activation(out=a_r, in_=ln_r, func=AF.Exp,
                                     scale=-0.5)
                d_r = rows.tile([1, NW], BF16, tag="r", bufs=6)
                nc.vector.scalar_tensor_tensor(
                    out=d_r, in0=mu_r, scalar=-1.0 / DIM,
                    in1=a_r, op0=ALU.mult, op1=ALU.mult)
                ad_ps = psum.tile([128, 2, NW], F32, tag="s", bufs=2)
                mm(ad_ps[:, 0, :], onecol, a_r, start=True, stop=True)
                mm(ad_ps[:, 1, :], onecol, d_r, start=True, stop=True)
                ad_sb = work.tile([128, 2, NW], BF16, name="ad_sb")
                nc.vector.tensor_copy(out=ad_sb, in_=ad_ps)
                state["ad_sb"] = ad_sb

            # ---- attention + interleaved LN-normalize / ctx2+out1 ----
            ot = work.tile([128, 4, SBATCH, 256], F8, name="ot")
            res = iop.tile([128, 4, SBATCH, 256], F32, name="res")

            def attn(c, j):
                s_ps = psum.tile([128, 2, NW], F32, tag="s", bufs=2)
                for h2 in range(2):
                    p0 = 64 * h2
                    for mc in range(2):
                        mm(s_ps[:, h2, mc * 256:(mc + 1) * 256],
                           kt[p0:p0 + 64, c, j, mc * 128:(mc + 1) * 128],
                           qt[p0:p0 + 64, c, j], start=True, stop=True)
                pt = ppool.tile([128, 2, 2, 256], BF16, tag="p", name="pt",
                                bufs=6)
                nc.scalar.activation(
                    out=pt.rearrange("p mc h2 n -> p h2 mc n"),
                    in_=s_ps.rearrange("p h2 (mc n) -> p h2 mc n", mc=2),
                    func=AF.Exp, scale=2.0 ** (-(kq + kk)))
                mbt = mbp.tile([128, 2, 2, 256], BF16, name="mbt")
                nc.sync.dma_start(
                    out=mbt,
                    in_=mbd[b0 + j, c].rearrange("mc h2 p n -> p mc h2 n"))
                nc.vector.tensor_mul(out=pt, in0=pt, in1=mbt)
                # reuse the scores tile's first bank for the denominator
                # (scores are dead after the exp); saves a PSUM bank per
                # in-flight iteration
                sbc = s_ps[:, 0, :]
                for mc in range(2):
                    mm(sbc, ones128,
                       pt[:, mc].rearrange("p h n -> p (h n)"),
                       start=mc == 0, stop=mc == 1)
                oo = psum.tile([64, 2, 256], F32, tag="oo", bufs=2)
                for h2 in range(2):
                    hd = (2 * c + h2) * 64
                    for mc in range(2):
                        mm(oo[:, h2, :], vt[:, j, mc, hd:hd + 64],
                           pt[:, mc, h2, :], start=mc == 0, stop=mc == 1)
                rec_sb = ppool.tile([128, NW], F32, tag="rb", name="rec_sb",
                                    bufs=4)
                nc.vector.reciprocal(out=rec_sb, in_=sbc)
                for h2 in range(2):
                    nc.vector.tensor_mul(
                        out=ot[h2 * 64:(h2 + 1) * 64, c, j],
                        in0=oo[:, h2, :],
                        in1=rec_sb[h2 * 64:(h2 + 1) * 64,
                                   h2 * 256:(h2 + 1) * 256])

            def normalize(c):
                # rl = relu(((h*a + d)) * (g*sqrt(512)) + b), written in place
                ad_sb = state["ad_sb"]
                nc.gpsimd.tensor_mul(out=ht[:, c], in0=ht[:, c],
                                     in1=ad_sb[:, 0, :])
                nc.gpsimd.tensor_add(out=ht[:, c], in0=ht[:, c],
                                     in1=ad_sb[:, 1, :])
                nc.scalar.activation(out=ht[:, c], in_=ht[:, c], func=AF.Relu,
                                     scale=lngc[:, c:c + 1],
                                     bias=lnbc[:, c:c + 1])  # Act (table: nlx)

            def ctx2wo(j):
                co = psum.tile([128, 2, NW], F32, tag="s", bufs=2)
                for f in range(4):
                    dst = co[:, f // 2, (f % 2) * 256:(f % 2) * 256 + 256]
                    for kc in range(4):
                        mm(dst, wb["wc2"][:, kc, f * 128:(f + 1) * 128],
                           ht[:, kc, j * 256:(j + 1) * 256],
                           start=kc == 0, stop=False)
                    for i in range(2):
                        mm(dst,
                           w8["wo"][:, 2 * i:2 * i + 2, f * 128:(f + 1) * 128],
                           ot[:, 2 * i:2 * i + 2, j, :],
                           start=False, stop=i == 1, perf_mode=DR)
                for f in range(4):
                    nc.scalar.activation(
                        out=res[:, f, j, :],
                        in_=co[:, f // 2, (f % 2) * 256:(f % 2) * 256 + 256],
                        func=AF.Identity, scale=2.0 ** (-g),
                        bias=bocbc[:, f:f + 1])
                nc.gpsimd.dma_start(
                    out=outT[b0 + j].rearrange("(c p) n -> p c n", p=128),
                    in_=res[:, :, j, :])

            attn(0, 0)
            ln_stats_a()
            attn(1, 0)
            ln_stats_b()
            attn(2, 0)
            normalize(0)
            normalize(1)
            attn(3, 0)
            normalize(2)
            normalize(3)
            ctx2wo(0)
            for c in range(4):
                attn(c, 1)
            ctx2wo(1)


def build(n_super, ks):
    # Pin the activation table: expose only natural_log_exp_and_others
    # (contains Exp/Ln/Relu/Identity/Copy/Square) to the act-table-load
    # insertion pass so it never flip-flops between sets.  Other entries are
    # emptied (not removed) to keep act_func_set_id indices valid.
    import concourse.bacc as bacc_mod
    from concourse.hw_specs import get_activation_tables as _gat
    def pinned_tables(arch):
        tabs = _gat(arch)
        return {name: (s if name == "natural_log_exp_and_others" else set())
                for name, s in tabs.items()}

    nc = bacc.Bacc("TRN2", target_bir_lowering=False, debug=False,
                   num_devices=N_CORES)
    dt = nc.dram_tensor
    io = (
        dt("x8", [BPC, DIM, N], F8, kind="ExternalInput").ap(),
        dt("c8", [BPC, DIM, N], F8, kind="ExternalInput").ap(),
        dt("cb", [BPC, DIM, N], BF16, kind="ExternalInput").ap(),
        dt("mb", [BPC, 4, 2, 2, 128, N], BF16, kind="ExternalInput").ap(),
        dt("wq", [DIM, DIM], F8, kind="ExternalInput").ap(),
        dt("wk", [DIM, DIM], F8, kind="ExternalInput").ap(),
        dt("wv", [DIM, DIM], F8, kind="ExternalInput").ap(),
        dt("wo", [DIM, DIM], F8, kind="ExternalInput").ap(),
        dt("wc1", [DIM, DIM], BF16, kind="ExternalInput").ap(),
        dt("wc2", [DIM, DIM], BF16, kind="ExternalInput").ap(),
        dt("lng", [DIM], F32, kind="ExternalInput").ap(),
        dt("lnb", [DIM], F32, kind="ExternalInput").ap(),
        dt("bc1", [DIM], F32, kind="ExternalInput").ap(),
        dt("bocb", [DIM], F32, kind="ExternalInput").ap(),
        dt("outT", [BPC, DIM, N], F32, kind="ExternalOutput").ap(),
    )
    with tile.TileContext(nc) as tc:
        _emit(nc, tc, io, n_super, ks)
    saved = bacc_mod.get_activation_tables
    bacc_mod.get_activation_tables = pinned_tables
    try:
        nc.compile()
    finally:
        bacc_mod.get_activation_tables = saved
    return nc


def _k_of(absmax):
    return int(math.floor(math.log2(120.0 / max(absmax, 1e-30))))


def prep_in_maps(x, context, mask, Wq, Wk, Wv, Wc1, bc1, ln_g, ln_b, Wc2, bc2,
                 Wo, bo, bias_table, rel_index):
    f = np.float32
    x = np.asarray(x, f)
    context = np.asarray(context, f)
    mask = np.asarray(mask)
    Wq = np.asarray(Wq, f) * SCALE
    Wk = np.asarray(Wk, f)
    Wv = np.asarray(Wv, f)
    Wo = np.asarray(Wo, f)
    Wc1 = np.asarray(Wc1, f)
    Wc2 = np.asarray(Wc2, f)

    kq = _k_of(np.abs(Wq).max())
    kk = _k_of(np.abs(Wk).max())
    kv = _k_of(np.abs(Wv).max())
    kwo = _k_of(np.abs(Wo).max())
    ks = (kq, kk, kv, kwo)
    g = OT_K + kwo

    xT = np.ascontiguousarray(
        x.reshape(N_CORES, BPC, N, DIM).transpose(0, 1, 3, 2))
    cT = np.ascontiguousarray(
        context.reshape(N_CORES, BPC, N, DIM).transpose(0, 1, 3, 2))
    x8 = xT.astype(NP8)
    c8 = cT.astype(NP8)
    cb = cT.astype(NPBF)

    # mb[core, b, c, mc, h2, p, n] = maskT[b, m, n] * exp(bias)[h, m, n]
    expBT = np.exp(
        np.asarray(bias_table, f)[np.asarray(rel_index)].transpose(2, 1, 0))
    # expBT: [H, m, n]; maskT: [core, b, m, n]
    mT = mask.reshape(N_CORES, BPC, N, N).transpose(0, 1, 3, 2).astype(f)
    mbf = mT[:, :, None, :, :] * expBT[None, None, :, :, :]  # [cr,b,h,m,n]
    mbf = mbf.reshape(N_CORES, BPC, 4, 2, 2, 128, N).transpose(
        0, 1, 2, 4, 3, 5, 6)  # [cr, b, c, mc, h2, p, n]
    mb = np.ascontiguousarray(mbf).astype(NPBF)

    shared = dict(
        wq=np.ascontiguousarray(Wq * 2.0 ** kq).astype(NP8),
        wk=np.ascontiguousarray(Wk * 2.0 ** kk).astype(NP8),
        wv=np.ascontiguousarray(Wv * 2.0 ** kv).astype(NP8),
        wo=np.ascontiguousarray(Wo * 2.0 ** kwo).astype(NP8),
        wc1=np.ascontiguousarray(Wc1).astype(NPBF),
        wc2=np.ascontiguousarray(Wc2 * 2.0 ** g).astype(NPBF),
        lng=np.ascontiguousarray(np.asarray(ln_g, f) * math.sqrt(DIM)),
        lnb=np.ascontiguousarray(np.asarray(ln_b, f)),
        bc1=np.ascontiguousarray(np.asarray(bc1, f)),
        bocb=np.ascontiguousarray(np.asarray(bo, f) + np.asarray(bc2, f)),
    )
    in_maps = [dict(x8=x8[c], c8=c8[c], cb=cb[c], mb=mb[c], **shared)
               for c in range(N_CORES)]
    return in_maps, ks


_nc_cache = {}


def _get_nc(n_super, ks):
    key = (n_super, ks)
    if key not in _nc_cache:
        _nc_cache[key] = build(n_super, ks)
    return _nc_cache[key]


def assemble_out(results):
    outT = np.stack([results[c]["outT"] for c in range(N_CORES)])
    return np.ascontiguousarray(
        outT.transpose(0, 1, 3, 2).reshape(B, N, DIM)).astype(np.float32)


def kernel(**inputs):
    in_maps, ks = prep_in_maps(**inputs)
    nc = _get_nc(NSUPER, ks)
    res = run_bass_kernel_spmd(nc, in_maps, core_ids=list(range(N_CORES)))
    return assemble_out(res.results)
